# revision 17
# baseline (speedup 1.0000x reference)
"""Trainium2 Bass kernel for nn_Decoder (GRU + Bahdanau attention + fc decoder).

Reference computation (B=64, POI=5000, EMB=256, UNITS=512, QDIM=256):
    x1       = concat(emb[x], query)                   [B, 512]
    output_  = GRUCell(x1, dec_hidden)                 [B, 512]   (keras, reset_after)
    v_proj   = emb @ W1_w + W1_b                       [POI, 512]
    q_proj   = output_ @ W2_w + W2_b                   [B, 512]
    score    = tanh(v_proj[None] + q_proj[:,None]) @ V_w (+V_b)   [B, POI]
    attn     = softmax(score, axis=1)
    context  = sum(attn * emb, axis=1)                 [B, EMB]
    logits   = concat(context, output_, cat_dec_hidden[0]) @ fc_w + fc_b
    returns (logits, state, output_)  with state == output_ == GRU h_new

Sharding over 8 cores:
  - GRU and v_proj^T: computed REPLICATED on every core (cheap; keeps all
    collectives off the critical path - the first collective costs ~40us).
  - attention tanh/score/softmax/context: BATCH-sharded (8 rows of B per
    core; selected via a one-hot `sel` input so the SPMD program is
    rank-agnostic). Softmax normalizer stays core-local (full POI per row).
  - context: AllGather [8,256] -> [64,256]  (the only collective).
  - fc: POI-column-sharded ([1280, 625] per core); host concatenates logits.

V_b is omitted: softmax is shift-invariant so it cannot affect any output.
A_hat is unused by the reference.

Numerics: fp32 except: GRU weights/x1/dec_hidden on the gate matmuls (bf16),
v_proj^T inputs + storage and tanh outputs (bf16, feeding the score dot with
bf16 V_w), exp-scores + emb on the context matmul (bf16), fc weights and its
lhsT operand (bf16). All PSUM accumulation, the GRU gate nonlinearity path,
h_new elementwise update (fp32 dec_hidden), softmax normalizer, biases and
outputs are fp32.
"""
import os
import sys

sys.path.insert(0, "/opt/trn_rl_repo")
os.environ.setdefault("MYCRO_LOCAL_CACHE", "1")

from contextlib import ExitStack

import numpy as np
import ml_dtypes

import concourse.bass as bass
import concourse.tile as tile
from concourse import bacc, mybir
from concourse.bass_utils import run_bass_kernel_spmd
from concourse.masks import make_identity

F32 = mybir.dt.float32
BF16 = mybir.dt.bfloat16
I32 = mybir.dt.int32
AF = mybir.ActivationFunctionType
ALU = mybir.AluOpType

NCORES = 8
POI, EMB, U, QDIM, B = 5000, 256, 512, 256, 64
PS = POI // NCORES          # 625   poi shard (fc columns)
BS = B // NCORES            # 8     batch shard
KIN = EMB + QDIM            # 512   GRU input dim
FCK = EMB + 2 * U           # 1280  fc contraction dim
PHL = POI // 2              # 2500  p-half (exp / transpose granularity)
# N-chunking of the full POI row for score matmuls (PSUM bank = 512 fp32)
SC_CH = [(j * 512, 512) for j in range(9)] + [(4608, POI - 4608)]
# N-chunking of the fc output shard
FC_CH = [(0, 512), (512, PS - 512)]

_CACHE = {}


def _build():
    nc = bacc.Bacc("TRN2", target_bir_lowering=False, debug=False,
                   num_devices=NCORES)

    def din(name, shape, dt=F32):
        return nc.dram_tensor(name, shape, dt, kind="ExternalInput").ap()

    emb_bf = din("emb_bf", [POI, EMB], BF16)        # gather source + ctx rhs
    embT_f = din("embT_f", [EMB, POI], BF16)        # emb^T, for v_proj rhs
    x_idx = din("x_idx", [B, 1], I32)
    queryT_bf = din("queryT_bf", [QDIM, B], BF16)
    dec_hT = din("dec_hT", [U, B])
    cat_hT_bf = din("cat_hT_bf", [U, B], BF16)
    gru_k_bf = din("gru_k_bf", [KIN, 3 * U], BF16)
    gru_r_bf = din("gru_r_bf", [U, 3 * U], BF16)
    gru_b0 = din("gru_b0", [3 * U, 1])
    gru_b1 = din("gru_b1", [3 * U, 1])
    w1 = din("w1", [EMB, U], BF16)
    w1b = din("w1b", [U, 1])
    w2 = din("w2", [U, U])
    w2b = din("w2b", [U, 1])
    vw = din("vw", [U, 1])
    fcw_s_bf = din("fcw_s_bf", [FCK, PS], BF16)
    fcb_s = din("fcb_s", [1, PS])
    sel = din("sel", [B, BS])

    logits_s = nc.dram_tensor("logits_s", [B, PS], F32, kind="ExternalOutput").ap()
    h_out = nc.dram_tensor("h_out", [B, U], F32, kind="ExternalOutput").ap()

    with tile.TileContext(nc) as tc, ExitStack() as ctx:
        sb = ctx.enter_context(tc.tile_pool(name="sb", bufs=1))
        tp = ctx.enter_context(tc.tile_pool(name="tpool", bufs=6))
        pp = ctx.enter_context(tc.tile_pool(name="pp", bufs=6, space="PSUM"))
        dr = ctx.enter_context(tc.tile_pool(name="dr", bufs=1, space="DRAM"))

        # ---- constants / ACT table prime / PE warm-up --------------------
        ident = sb.tile([128, 128], F32)
        make_identity(nc, ident[:])
        ident_bf = sb.tile([128, 128], BF16)
        nc.vector.tensor_copy(ident_bf[:], ident[:])
        prime = sb.tile([1, 8], F32)
        nc.vector.memset(prime[:], 0.0)
        prime2 = sb.tile([1, 8], F32)
        # force the exp_and_others ACT table (has exp AND tanh) to load early
        nc.scalar.activation(prime2[:], prime[:], AF.Exp)
        ones1 = sb.tile([1, B], F32)
        nc.vector.memset(ones1[:], 1.0)
        # ~4.5us of junk matmuls: trip the PE HAM activity monitor to K=8/8
        # (2.4 GHz) before the real front matmuls arrive.
        junk_src = sb.tile([128, 512], F32)
        nc.vector.memset(junk_src[:], 1e-5)
        junk_ps = pp.tile([128, 512], F32, tag="p", name="junk_ps")
        for i in range(10):
            nc.tensor.matmul(junk_ps[:], ident[:], junk_src[:],
                             start=(i == 0), stop=(i == 9))
        nc.vector.tensor_copy(junk_src[:, 0:1], junk_ps[:, 0:1])  # keep alive

        # ---- input DMAs, critical-path order, one DMA per tensor ---------
        def load_chunked(name, src, rows, cols, dt=F32):
            """DRAM [rows, cols] -> SBUF [128, (rows/128)*cols] in one DMA."""
            nch = rows // 128
            t = sb.tile([128, nch * cols], dt, name=name)
            nc.sync.dma_start(
                out=t[:].rearrange("p (k c) -> p k c", k=nch),
                in_=src.rearrange("(k p) c -> p k c", p=128))
            return t

        x_sb = sb.tile([B, 1], I32)
        nc.sync.dma_start(out=x_sb[:], in_=x_idx[:])
        b0_sb = sb.tile([128, 12], F32)   # col g*4+m = bias0 chunk [128,1]
        nc.sync.dma_start(out=b0_sb[:].rearrange("p (c o) -> p c o", c=12),
                          in_=gru_b0.rearrange("(c p) o -> p c o", p=128))
        b1_sb = sb.tile([128, 12], F32)
        nc.sync.dma_start(out=b1_sb[:].rearrange("p (c o) -> p c o", c=12),
                          in_=gru_b1.rearrange("(c p) o -> p c o", p=128))
        sel_sb = sb.tile([B, BS], F32)
        nc.sync.dma_start(out=sel_sb[:], in_=sel[:])
        qT_sb = load_chunked("qT_sb", queryT_bf, QDIM, B, dt=BF16)
        dhT_sb = load_chunked("dhT_sb", dec_hT, U, B)
        dhT_bf = sb.tile([128, 4 * B], BF16)
        nc.vector.tensor_copy(dhT_bf[:], dhT_sb[:])
        # GRU weights streamed per k-chunk (bf16)
        gkt, grt = [], []
        for k in range(4):
            g1 = tp.tile([128, 3 * U], BF16, name="gkt", bufs=4)
            nc.sync.dma_start(out=g1[:], in_=gru_k_bf[k * 128:(k + 1) * 128, :])
            gkt.append(g1)
        for k in range(4):
            g2 = tp.tile([128, 3 * U], BF16, name="grt", bufs=4)
            nc.sync.dma_start(out=g2[:], in_=gru_r_bf[k * 128:(k + 1) * 128, :])
            grt.append(g2)
        w1_sb = load_chunked("w1_sb", w1, EMB, U, dt=BF16)
        # embT streamed per 512-col block for v_proj (both k-chunks per DMA)
        et_tiles = []
        for (n0, nl) in SC_CH:
            et = tp.tile([128, 2 * 512], BF16, name="et", bufs=2)
            nc.sync.dma_start(
                out=et[:, :2 * nl].rearrange("p (k c) -> p k c", k=2),
                in_=embT_f[:, n0:n0 + nl].rearrange("(k p) c -> p k c", p=128))
            et_tiles.append(et)
        w2_sb = load_chunked("w2_sb", w2, U, U)
        wbv_sb = sb.tile([128, 12], F32)
        for i, t in enumerate((w1b, w2b, vw)):
            nc.sync.dma_start(
                out=wbv_sb[:, 4 * i:4 * (i + 1)].rearrange("p (k o) -> p k o", k=4),
                in_=t.rearrange("(k p) o -> p k o", p=128))
        w1b_sb, w2b_sb, vw_sb = wbv_sb[:, 0:4], wbv_sb[:, 4:8], wbv_sb[:, 8:12]
        vw_bf = sb.tile([128, 4], BF16)
        nc.vector.tensor_copy(vw_bf[:], vw_sb)
        chT_sb = load_chunked("chT_sb", cat_hT_bf, U, B, dt=BF16)
        fcb_sb = sb.tile([1, PS], F32)
        nc.sync.dma_start(out=fcb_sb[:], in_=fcb_s[:])
        fcw_sb = load_chunked("fcw_sb", fcw_s_bf, FCK, PS, dt=BF16)

        # ---- GRU, replicated, transposed layout; m-chunks as psum columns
        embx = sb.tile([B, EMB], BF16)
        nc.gpsimd.indirect_dma_start(
            out=embx[:], out_offset=None, in_=emb_bf[:],
            in_offset=bass.IndirectOffsetOnAxis(ap=x_sb[:, 0:1], axis=0))

        x1T_bf = sb.tile([128, 4 * B], BF16)  # x1^T = [emb[x];query]^T (bf16)
        for c in range(2):
            tpx = pp.tile([128, B], BF16, tag="p", name="tpx")
            nc.tensor.transpose(out=tpx[:], in_=embx[:, c * 128:(c + 1) * 128],
                                identity=ident_bf[:B, :B])
            nc.vector.tensor_copy(x1T_bf[:, c * B:(c + 1) * B], tpx[:])
        nc.vector.tensor_copy(x1T_bf[:, 2 * B:4 * B], qT_sb[:, 0:2 * B])

        # one psum tile per (gate-part, m-chunk): a start=True clears
        # has_written for the WHOLE bank, so regions cannot share a bank
        # across accumulation groups.
        def gru_mms(g, m, use_x, use_r):
            ps = pp.tile([128, B], F32, tag="p", name=f"gps{g}")
            parts = ([(gkt, x1T_bf)] if use_x else []) + \
                    ([(grt, dhT_bf)] if use_r else [])
            n = 4 * len(parts)
            i = 0
            for wt, rhs in parts:
                for k in range(4):
                    nc.tensor.matmul(
                        ps[:], wt[k][:, g * U + m * 128:g * U + (m + 1) * 128],
                        rhs[:, k * B:(k + 1) * B],
                        start=(i == 0), stop=(i == n - 1))
                    i += 1
            return ps
        # alloc order matters for pool rotation: xh tiles are long-lived
        # (read only at cpre), so allocate them LAST to avoid a slot cycle
        ps_z = [gru_mms(0, m, True, True) for m in range(4)]
        ps_r = [gru_mms(1, m, True, True) for m in range(4)]
        ps_hh = [gru_mms(2, m, False, True) for m in range(4)]
        ps_xh = [gru_mms(2, m, True, False) for m in range(4)]

        # biases: z/r combined half-bias (sigmoid(x) = 0.5 + 0.5*tanh(x/2))
        ball = sb.tile([128, 12], F32)
        nc.vector.tensor_add(ball[:], b0_sb[:], b1_sb[:])
        bzr_h = sb.tile([128, 8], F32)
        nc.vector.tensor_scalar_mul(bzr_h[:], ball[:, 0:8], 0.5)

        zt = sb.tile([128, 256], F32)
        rt = sb.tile([128, 256], F32)
        hh = sb.tile([128, 256], F32)
        for m in range(4):
            nc.scalar.activation(zt[:, m * 64:(m + 1) * 64], ps_z[m][:], AF.Tanh,
                                 bias=bzr_h[:, m:m + 1], scale=0.5)
            nc.scalar.activation(rt[:, m * 64:(m + 1) * 64], ps_r[m][:], AF.Tanh,
                                 bias=bzr_h[:, 4 + m:5 + m], scale=0.5)
            nc.scalar.add(hh[:, m * 64:(m + 1) * 64],
                          ps_hh[m][:], b1_sb[:, 8 + m:9 + m])
        m1 = sb.tile([128, 256], F32)
        nc.vector.tensor_mul(m1[:], rt[:], hh[:])
        m2 = sb.tile([128, 256], F32)
        nc.vector.tensor_add(m2[:], hh[:], m1[:])
        # c_pre = xh + 0.5*(hh + rt*hh)   (r*hh with r = 0.5+0.5*rt)
        cpre = sb.tile([128, 256], F32)
        for m in range(4):
            nc.vector.scalar_tensor_tensor(cpre[:, m * 64:(m + 1) * 64],
                                           in0=m2[:, m * 64:(m + 1) * 64],
                                           scalar=0.5, in1=ps_xh[m][:],
                                           op0=ALU.mult, op1=ALU.add)
        ct = sb.tile([128, 256], F32)
        for m in range(4):
            nc.scalar.activation(ct[:, m * 64:(m + 1) * 64],
                                 cpre[:, m * 64:(m + 1) * 64], AF.Tanh,
                                 bias=b0_sb[:, 8 + m:9 + m], scale=1.0)
        # h_new = 0.5*((h_prev + c) + zt*(h_prev - c)), fp32 h_prev
        s_ = sb.tile([128, 256], F32)
        nc.vector.tensor_add(s_[:], dhT_sb[:], ct[:])
        d_ = sb.tile([128, 256], F32)
        nc.vector.tensor_sub(d_[:], dhT_sb[:], ct[:])
        m_ = sb.tile([128, 256], F32)
        nc.vector.tensor_mul(m_[:], zt[:], d_[:])
        hn2 = sb.tile([128, 256], F32)
        nc.vector.tensor_add(hn2[:], s_[:], m_[:])
        hT_sb = sb.tile([128, 4 * B], F32)     # full h_new^T, all 64 b
        nc.vector.tensor_scalar_mul(hT_sb[:], hn2[:], 0.5)
        hT_bf = sb.tile([128, 4 * B], BF16)    # bf16 copy for the fc lhsT
        nc.vector.tensor_copy(hT_bf[:], hT_sb[:])

        # ---- v_proj^T computed fully on every core (bf16 result) ---------
        vfull = sb.tile([128, 4 * POI], BF16)
        for j, (n0, nl) in enumerate(SC_CH):
            for m in range(4):
                ps_v = pp.tile([128, 512], F32, tag="p", name="ps_v")
                for k in range(2):
                    nc.tensor.matmul(ps_v[:, :nl],
                                     w1_sb[:, k * U + m * 128:k * U + (m + 1) * 128],
                                     et_tiles[j][:, k * nl:(k + 1) * nl],
                                     start=(k == 0), stop=(k == 1))
                nc.vector.tensor_copy(vfull[:, m * POI + n0:m * POI + n0 + nl],
                                      ps_v[:, :nl])

        # h (non-transposed) for the state output + sel matmul
        h_sb = sb.tile([B, U], F32)
        for k in range(4):
            tph = pp.tile([B, 128], F32, tag="p", name="tph")
            nc.tensor.transpose(out=tph[:], in_=hT_sb[:, k * B:(k + 1) * B],
                                identity=ident[:, :])
            nc.vector.tensor_copy(h_sb[:, k * 128:(k + 1) * 128], tph[:])
        nc.sync.dma_start(out=h_out[:], in_=h_sb[:])

        # my 8 batch rows of h via one-hot selection (keeps program rank-agnostic)
        ps_hm = pp.tile([BS, U], F32, tag="p", name="ps_hm")
        nc.tensor.matmul(ps_hm[:], sel_sb[:], h_sb[:], start=True, stop=True)
        hm_sb = sb.tile([BS, U], F32)
        nc.vector.tensor_copy(hm_sb[:], ps_hm[:])
        hTm_sb = sb.tile([128, 4 * BS], F32)
        for k in range(4):
            tpm = pp.tile([128, BS], F32, tag="p", name="tpm")
            nc.tensor.transpose(out=tpm[:], in_=hm_sb[:, k * 128:(k + 1) * 128],
                                identity=ident[:BS, :BS])
            nc.vector.tensor_copy(hTm_sb[:, k * BS:(k + 1) * BS], tpm[:])

        # biasT[u, j] = q_proj^T + W1_b + W2_b   for my 8 batch rows
        bv_sb = sb.tile([128, 4], F32)
        nc.vector.tensor_add(bv_sb[:], w1b_sb, w2b_sb)
        biasT = sb.tile([128, 4 * BS], F32)
        for m in range(4):
            ps_q = pp.tile([128, BS], F32, tag="p", name="ps_q")
            for k in range(4):
                nc.tensor.matmul(ps_q[:], w2_sb[:, k * U + m * 128:k * U + (m + 1) * 128],
                                 hTm_sb[:, k * BS:(k + 1) * BS],
                                 start=(k == 0), stop=(k == 3))
            nc.scalar.add(biasT[:, m * BS:(m + 1) * BS], ps_q[:], bv_sb[:, m:m + 1])

        # ---- attention main loop (ACT-bound) -----------------------------
        # tanh in [128, 5000] tiles: 4 ACT instructions per batch row.
        # Wave A: chunks 0-4 accumulate per-tanh (keeps PE fed).
        # Wave B: chunks 5-9 chunk-major after all 4 tanhs (long warm PE
        # burst, early PSUM evac keeps <= 3 score banks live).
        scores = sb.tile([BS, POI], F32)
        n_ph = sb.tile([BS, 2], F32)
        for bi in range(BS):
            t_tiles = []
            scs_a = [pp.tile([1, 512], F32, tag="p", name=f"sa{j}")
                     for j in range(5)]
            for m in range(4):
                t_bf = tp.tile([128, POI], BF16, tag="T", name="t_bf")
                t_tiles.append(t_bf)
                nc.scalar.activation(
                    t_bf[:], vfull[:, m * POI:(m + 1) * POI],
                    AF.Tanh, bias=biasT[:, m * BS + bi:m * BS + bi + 1])
                for j in range(5):
                    n0, nl = SC_CH[j]
                    nc.tensor.matmul(scs_a[j][:, :nl], vw_bf[:, m:m + 1],
                                     t_bf[:, n0:n0 + nl],
                                     start=(m == 0), stop=(m == 3))
            for j in range(5):
                n0, nl = SC_CH[j]
                st = tp.tile([1, 512], F32, name="st", bufs=3)
                nc.vector.tensor_copy(st[:1, :nl], scs_a[j][:1, :nl])
                q = nc.sync if (bi + j) % 2 == 0 else nc.gpsimd
                q.dma_start(out=scores[bi:bi + 1, n0:n0 + nl], in_=st[:1, :nl])
            for j in range(5, 10):
                n0, nl = SC_CH[j]
                ps_sc = pp.tile([1, 512], F32, tag="p", name="ps_sc")
                for m in range(4):
                    nc.tensor.matmul(ps_sc[:, :nl], vw_bf[:, m:m + 1],
                                     t_tiles[m][:, n0:n0 + nl],
                                     start=(m == 0), stop=(m == 3))
                st = tp.tile([1, 512], F32, name="st", bufs=3)
                nc.vector.tensor_copy(st[:1, :nl], ps_sc[:1, :nl])
                q = nc.sync if (bi + j) % 2 == 0 else nc.gpsimd
                q.dma_start(out=scores[bi:bi + 1, n0:n0 + nl], in_=st[:1, :nl])

        # exp in place per p-half; accumulate the softmax normalizer
        expT = sb.tile([128, 40 * BS], BF16)
        for ph in range(2):
            nc.scalar.activation(scores[:, ph * PHL:(ph + 1) * PHL],
                                 scores[:, ph * PHL:(ph + 1) * PHL],
                                 AF.Exp, accum_out=n_ph[:, ph:ph + 1])
            # transpose this half's exp scores into [128, 8] chunks (bf16)
            for c in range(20 * ph, 20 * (ph + 1)):
                w = 128 if c < 39 else POI - 39 * 128  # 8-wide tail chunk
                tpe = pp.tile([128, BS], F32, tag="p", name="tpe")
                nc.tensor.transpose(out=tpe[:w, :],
                                    in_=scores[:, c * 128:c * 128 + w],
                                    identity=ident[:BS, :BS])
                nc.vector.tensor_copy(expT[:w, c * BS:(c + 1) * BS], tpe[:w, :])

        n_sb = sb.tile([BS, 1], F32)
        nc.vector.tensor_add(n_sb[:], n_ph[:, 0:1], n_ph[:, 1:2])
        rn_sb = sb.tile([BS, 1], F32)
        nc.vector.reciprocal(rn_sb[:], n_sb[:])

        # context (unnormalized): ctx[j, e] = sum_p exp[j, p] * emb[p, e]
        # emb rhs streamed from DRAM (bf16) through a small rotating pool
        ps_ctx = pp.tile([BS, EMB], F32, tag="p", name="ps_ctx")
        for c in range(40):
            w = 128 if c < 39 else POI - 39 * 128
            ec = tp.tile([128, EMB], BF16, name="ec", bufs=6)
            nc.sync.dma_start(out=ec[:w, :], in_=emb_bf[c * 128:c * 128 + w, :])
            nc.tensor.matmul(ps_ctx[:], expT[:w, c * BS:(c + 1) * BS],
                             ec[:w, :], start=(c == 0), stop=(c == 39))
        ctx_sb = sb.tile([BS, EMB], F32)
        nc.vector.tensor_scalar_mul(ctx_sb[:], ps_ctx[:], rn_sb[:, 0:1])

        # ---- fc partials (bias + h + cat chunks) BEFORE the ctx AllGather,
        # so these matmuls execute on PE while the collective runs.
        ps_fcs = []
        for (n0, nl) in FC_CH:
            ps_fc = pp.tile([B, 512], F32, tag="fc", name="ps_fc", bufs=2)
            ps_fcs.append(ps_fc)
            nc.tensor.matmul(ps_fc[:, :nl], ones1[:, :], fcb_sb[:, n0:n0 + nl],
                             start=True, stop=False)
            for i in range(8):   # h chunks (fc rows 256:768), cat (768:1280)
                kr = 2 + i
                src = hT_bf if i < 4 else chT_sb
                lhsT = src[:, (i % 4) * B:(i % 4 + 1) * B]
                nc.tensor.matmul(ps_fc[:, :nl], lhsT,
                                 fcw_sb[:, kr * PS + n0:kr * PS + n0 + nl],
                                 start=False, stop=False)

        ag3_in = dr.tile([BS, EMB], F32)
        ag3_out = dr.tile([B, EMB], F32, addr_space="Shared")
        nc.sync.dma_start(out=ag3_in[:], in_=ctx_sb[:])
        nc.gpsimd.collective_compute(
            "AllGather", ALU.bypass, replica_groups=[list(range(NCORES))],
            ins=[ag3_in[:]], outs=[ag3_out[:]])
        ctxf = sb.tile([B, EMB], F32)
        nc.sync.dma_start(out=ctxf[:], in_=ag3_out[:])

        ctxT = sb.tile([128, 2 * B], BF16)
        for k in range(2):
            tpc = pp.tile([128, B], F32, tag="p", name="tpc")
            nc.tensor.transpose(out=tpc[:], in_=ctxf[:, k * 128:(k + 1) * 128],
                                identity=ident[:B, :B])
            nc.vector.tensor_copy(ctxT[:, k * B:(k + 1) * B], tpc[:])

        # finish fc with the two ctx chunks (fc rows 0:256), evac, store
        for (n0, nl), ps_fc in zip(FC_CH, ps_fcs):
            for k in range(2):
                nc.tensor.matmul(ps_fc[:, :nl], ctxT[:, k * B:(k + 1) * B],
                                 fcw_sb[:, k * PS + n0:k * PS + n0 + nl],
                                 start=False, stop=(k == 1))
            lg = sb.tile([B, 512], F32, name="lg")
            nc.vector.tensor_copy(lg[:, :nl], ps_fc[:, :nl])
            nc.sync.dma_start(out=logits_s[:, n0:n0 + nl], in_=lg[:, :nl])

    nc.compile()
    return nc


def _prep_inputs(inputs):
    """Host-side sharding / layout prep (data movement + dtype casts only)."""
    f = lambda a: np.ascontiguousarray(np.asarray(a), dtype=np.float32)
    bf = ml_dtypes.bfloat16
    emb = f(inputs["poi_embedding"])
    query = f(inputs["query"])
    dec_h = f(inputs["dec_hidden"])
    cat_h = f(inputs["cat_dec_hidden"])[0]
    gk = f(inputs["gru_kernel"])
    gr = f(inputs["gru_rec_kernel"])
    gb = f(inputs["gru_bias"])
    w1 = f(inputs["W1_w"]); w1b = f(inputs["W1_b"]).reshape(U, 1)
    w2 = f(inputs["W2_w"]); w2b = f(inputs["W2_b"]).reshape(U, 1)
    vw = f(inputs["V_w"]).reshape(U, 1)
    fcw = f(inputs["fc_w"]); fcb = f(inputs["fc_b"]).reshape(1, POI)
    x = np.asarray(inputs["x"]).astype(np.int32).reshape(B, 1)

    emb_bf = emb.astype(bf)
    embT_bf = np.ascontiguousarray(emb.T).astype(bf)
    queryT_bf = np.ascontiguousarray(query.T).astype(bf)
    dec_hT = np.ascontiguousarray(dec_h.T)
    cat_hT_bf = np.ascontiguousarray(cat_h.T).astype(bf)

    in_maps = []
    for c in range(NCORES):
        selm = np.zeros((B, BS), np.float32)
        selm[c * BS + np.arange(BS), np.arange(BS)] = 1.0
        in_maps.append({
            "emb_bf": emb_bf,
            "embT_f": embT_bf,
            "x_idx": x,
            "queryT_bf": queryT_bf,
            "dec_hT": dec_hT,
            "cat_hT_bf": cat_hT_bf,
            "gru_k_bf": gk.astype(bf),
            "gru_r_bf": gr.astype(bf),
            "gru_b0": np.ascontiguousarray(gb[0]).reshape(3 * U, 1),
            "gru_b1": np.ascontiguousarray(gb[1]).reshape(3 * U, 1),
            "w1": w1.astype(bf), "w1b": w1b, "w2": w2, "w2b": w2b, "vw": vw,
            "fcw_s_bf": np.ascontiguousarray(fcw[:, c * PS:(c + 1) * PS]).astype(bf),
            "fcb_s": np.ascontiguousarray(fcb[:, c * PS:(c + 1) * PS]),
            "sel": selm,
        })
    return in_maps


def kernel(**inputs):
    if "nc" not in _CACHE:
        _CACHE["nc"] = _build()
    nc = _CACHE["nc"]
    in_maps = _prep_inputs(inputs)
    res = run_bass_kernel_spmd(nc, in_maps, list(range(NCORES)))
    r = res.results
    logits = np.concatenate([r[c]["logits_s"] for c in range(NCORES)], axis=1)
    h_new = r[0]["h_out"]
    return (logits, h_new, h_new.copy())
